# revision 34
# baseline (speedup 1.0000x reference)
"""Trainium2 Bass kernel for nn_CESLayer: y = cos((x+1)*30 @ theta.T + phi).

Math: (x+1)*30 @ theta.T + phi = x @ W + bias, with
  W[k, o] = 30 * theta[o, k],  bias[o] = 30 * sum_k theta[o, k] + phi[o]
and cos(z) = sin(z + pi/2). The ScalarE Sin LUT is only valid on [-pi, pi],
so the kernel computes u = z / (2*pi) via a rescaled matmul (W' = W/2pi),
adds the (mod-1-reduced) bias b while converting PSUM->SBUF f16,
  h = u + b            (f16, |h| <~ 3)
range-reduces with the f32 magic-number trick on cheap all-f16 DVE ops,
  n = (h + MAGIC) - MAGIC   (one 2-ALU tensor_scalar; exact small integer)
  f = h - n                 (in [-0.5, 0.5])
and evaluates Sin(2*pi*f) on ScalarE. The bias rides per-partition scalar
operands so there are NO rank-1 bias matmuls on PE (saves 128 matmuls).

The PSUM->SBUF h step alternates between DVE (tensor_scalar add) and
ScalarE (Identity activation with per-partition bias) per output tile,
balancing DVE and ACT busy time under the PE matmul roofline.

Layout: the output is computed TRANSPOSED (psum tiles are [o, b]) so the
per-output scalars ride per-partition operands; the host un-transposes.
x is pre-transposed host-side into [k, b] super-group tiles of 4 batch
groups; each (gs, j) load is one [128, 2048] 512KB DMA. Matmuls run
j-outer/s-inner so consecutive instructions share the stationary operand
(LDWEIGHTS overlaps the previous matmul's streaming); each (ob, gs) uses
one 4-bank [128, 2048] PSUM tile so the epilogue runs few, wide ops.
The first super-group interleaves w and x block loads j-first so the
first matmul issues ~10us in; the last super-group runs narrow 1024-wide
all-DVE epilogue strips to shorten the drain. Batch is split across 8
NeuronCores (data parallel), weights replicated.

Matmuls run in fp16 (full PE rate vs 4x slower fp32) with fp32 PSUM
accumulation; the output is stored fp16 (upcast on host), halving write
traffic. fp16 quantization contributes ~7e-4 relative error.
"""

import os
import sys

for _p in (
    "/root/.axon_site",
    "/root/.axon_site/_ro/trn_rl_repo",
    "/root/.axon_site/_ro/pypackages",
    "/opt/trn_rl_repo",
):
    if os.path.isdir(_p) and _p not in sys.path:
        sys.path.append(_p)

import ml_dtypes
import numpy as np

OMEGA_0 = 30.0
B, IN_DIM, OUT_DIM = 131072, 512, 512
N_CORES = 8
BS = B // N_CORES  # rows per core
P = 128  # partitions
KB = IN_DIM // P  # contraction blocks
OB = OUT_DIM // P  # output blocks
GW = 512  # batch columns per matmul (moving free dim, one PSUM bank)
SG = 4  # batch groups per DMA super-group
GROUPS = BS // GW
SGROUPS = GROUPS // SG
SGW = SG * GW
TWO_PI = 2.0 * np.pi
MAGIC = float(np.float32(1.5 * 2**23))  # f32 round-to-nearest via add/sub

CONFIG = {
    "mm_dt": os.environ.get("K_MM_DT", "f16"),
    "out_dt": os.environ.get("K_OUT_DT", "f16"),
}

_cache = {}


def _np_dt(name):
    return {
        "f16": np.float16,
        "bf16": ml_dtypes.bfloat16,
        "f32": np.float32,
        "f32r": np.float32,
    }[name]


def _build(sgroups=SGROUPS, num_devices=N_CORES, cfg=None):
    import concourse.mybir as mybir
    import concourse.tile as tile
    from concourse import bacc

    cfg = dict(CONFIG if cfg is None else cfg)
    f32 = mybir.dt.float32
    f16 = mybir.dt.float16
    mm_dt = {
        "f16": mybir.dt.float16,
        "bf16": mybir.dt.bfloat16,
        "f32r": mybir.dt.float32r,
    }[cfg["mm_dt"]]
    out_dt = {"f16": mybir.dt.float16, "f32": mybir.dt.float32}[cfg["out_dt"]]
    Alu = mybir.AluOpType
    Act = mybir.ActivationFunctionType

    nc = bacc.Bacc(
        "TRN2",
        target_bir_lowering=False,
        debug=False,
        enable_asserts=False,
        num_devices=num_devices,
    )
    # xt[gs, k, s*GW + b] = x[(gs*SG + s)*GW + b, k]
    xt_d = nc.dram_tensor(
        "xt", [sgroups, IN_DIM, SGW], mm_dt, kind="ExternalInput"
    ).ap()
    # w[k, o] = 30 * theta[o, k] / (2*pi)
    w_d = nc.dram_tensor("w", [IN_DIM, OUT_DIM], mm_dt, kind="ExternalInput").ap()
    # sadd[p, ob] = mod-1-reduced bias[ob*P + p]/2pi; sadd[p, OB] = 0 (Sin bias)
    sadd_d = nc.dram_tensor("sadd", [P, OB + 1], f32, kind="ExternalInput").ap()
    # yt[ob, gs, p, s*GW + b] = y[(gs*SG + s)*GW + b, ob*P + p]
    yt_d = nc.dram_tensor(
        "yt", [OB, sgroups, P, SGW], out_dt, kind="ExternalOutput"
    ).ap()

    with tile.TileContext(nc) as tc:
        with (
            tc.tile_pool(name="const", bufs=1) as cpool,
            tc.tile_pool(name="xin", bufs=8) as xpool,
            tc.tile_pool(name="eps", bufs=4) as epool,
            tc.tile_pool(name="yout", bufs=2) as ypool,
            tc.tile_pool(name="psumM", bufs=2, space="PSUM") as pMpool,
        ):
            # head: interleave w and first-supergroup x loads j-first so the
            # j=0 matmuls can start as soon as the first two DMAs land
            w_sb = cpool.tile([P, KB * OUT_DIM], mm_dt)
            xg0 = xpool.tile([P, KB * SGW], mm_dt, tag="xg", name="xg0")
            # x block first: its transfer is the long pole for the first
            # matmul; w (128KB) catches up during it. Split j=0 so the s=0/1
            # matmuls can start after a 256KB transfer.
            nc.sync.dma_start(
                xg0[:, : SGW // 2], xt_d[0, :P, : SGW // 2]
            )
            nc.sync.dma_start(
                w_sb[:, :OUT_DIM], w_d[:P, :]
            )
            nc.sync.dma_start(
                xg0[:, SGW // 2 : SGW], xt_d[0, :P, SGW // 2 :]
            )
            for j in range(1, KB):
                nc.sync.dma_start(
                    xg0[:, j * SGW : (j + 1) * SGW],
                    xt_d[0, j * P : (j + 1) * P, :],
                )
                nc.sync.dma_start(
                    w_sb[:, j * OUT_DIM : (j + 1) * OUT_DIM],
                    w_d[j * P : (j + 1) * P, :],
                )
            sadd_sb = cpool.tile([P, OB + 1], f32)
            nc.sync.dma_start(sadd_sb[:], sadd_d[:])
            # PE p-state warmup: the PE ramps to full clock only after ~3us
            # of continuous activity. Stream discarded matmuls on memset data
            # while the first DMAs are in flight so the real matmuls start at
            # full speed. Results land in the first psum tile and are wiped
            # by its first real start=True matmul.
            warm_sb = cpool.tile([P, GW], f16)
            nc.gpsimd.memset(warm_sb[:], 0.0)
            warm_pm = pMpool.tile([P, 2 * GW], f32, tag="pm0", name="warm_pm")
            for _ in range(8):
                nc.tensor.matmul(
                    warm_pm[:, :GW],
                    warm_sb[:, :P],
                    warm_sb[:],
                    start=True,
                    stop=True,
                    skip_group_check=True,
                )

            strip_idx = 0
            for gs in range(sgroups):
                if gs == 0:
                    xg = xg0
                else:
                    xg = xpool.tile([P, KB * SGW], mm_dt, tag="xg", name=f"xg{gs}")
                    for j in range(KB):
                        nc.sync.dma_start(
                            xg[:, j * SGW : (j + 1) * SGW],
                            xt_d[gs, j * P : (j + 1) * P, :],
                        )
                last_gs = gs == sgroups - 1
                ysw = [
                    ypool.tile(
                        [P, SGW], out_dt, tag=f"ys{ob}", name=f"ysw{ob}_{gs}"
                    )
                    for ob in range(OB)
                ]
                HP = SG // 2
                for ob in range(OB):
                    # j-outer / s-inner: consecutive matmuls share the same
                    # stationary operand so LDWEIGHTS hides under the previous
                    # matmul's streaming. s-pairs share one 2-bank [P, 2*GW]
                    # psum tile so the epilogue runs fewer, wider ops.
                    pms = [
                        pMpool.tile(
                            [P, 2 * GW], f32, tag=f"pm{h}", name=f"pm{ob}_{gs}_{h}"
                        )
                        for h in range(HP)
                    ]
                    for j in range(KB):
                        for s in range(SG):
                            nc.tensor.matmul(
                                pms[s // 2][:, (s % 2) * GW : (s % 2 + 1) * GW],
                                w_sb[
                                    :,
                                    j * OUT_DIM + ob * P : j * OUT_DIM + (ob + 1) * P,
                                ],
                                xg[:, j * SGW + s * GW : j * SGW + (s + 1) * GW],
                                start=(j == 0),
                                stop=(j == KB - 1),
                                skip_group_check=True,
                            )
                    b_vec = sadd_sb[:, ob : ob + 1]
                    zero_vec = sadd_sb[:, OB : OB + 1]
                    last_obgs = last_gs and ob == OB - 1
                    for h in range(HP):
                        pm = pms[h]
                        ys = ysw[ob][:, h * 2 * GW : (h + 1) * 2 * GW]
                        if last_gs:
                            # drain balance: split the final supergroup's
                            # epilogue evenly between DVE and ScalarE
                            use_act = strip_idx % 2 == 1
                        else:
                            use_act = (strip_idx * 3) % 8 < 3
                        strip_idx += 1
                        h_t = epool.tile([P, 2 * GW], f16, tag="hf")
                        n_t = epool.tile([P, 2 * GW], f16, tag="nf")
                        f_t = epool.tile([P, 2 * GW], f16, tag="ff")
                        chunks = 1
                        cw = 2 * GW // chunks
                        for c in range(chunks):
                            sl = slice(c * cw, (c + 1) * cw)
                            if use_act:
                                # ScalarE path: Identity LUT applies the bias
                                # during the PSUM->SBUF f16 conversion
                                nc.scalar.activation(
                                    h_t[:, sl],
                                    pm[:, sl],
                                    Act.Identity,
                                    bias=b_vec,
                                    scale=1.0,
                                )
                            else:
                                nc.vector.tensor_scalar(
                                    h_t[:, sl], pm[:, sl], b_vec, None, Alu.add
                                )
                            # all-f16 range reduction: n = rtne(h) (exact),
                            # then f = h - n in [-0.5, 0.5]
                            nc.vector.tensor_scalar(
                                n_t[:, sl],
                                h_t[:, sl],
                                MAGIC,
                                MAGIC,
                                Alu.add,
                                Alu.subtract,
                            )
                            nc.vector.tensor_tensor(
                                f_t[:, sl], h_t[:, sl], n_t[:, sl], Alu.subtract
                            )
                            nc.scalar.activation(
                                ys[:, sl],
                                f_t[:, sl],
                                Act.Sin,
                                scale=float(TWO_PI),
                                bias=zero_vec,
                            )
                    # ship the strip while later strips compute
                    nc.sync.dma_start(yt_d[ob, gs], ysw[ob][:])

    nc.compile()
    return nc


def _get_nc():
    if "nc" not in _cache:
        _cache["nc"] = _build()
    return _cache["nc"]


def _host_params(theta, phi, cfg=None):
    cfg = dict(CONFIG if cfg is None else cfg)
    mm_np = _np_dt(cfg["mm_dt"])
    w = np.ascontiguousarray(
        (OMEGA_0 / TWO_PI) * theta.T.astype(np.float64)
    ).astype(mm_np)
    bias = (
        OMEGA_0 * theta.astype(np.float64).sum(axis=1) + phi + np.pi / 2
    ) / TWO_PI
    bias_red = bias - np.round(bias)  # in [-0.5, 0.5]: keeps |h| small (f16)
    sadd = np.concatenate(
        [
            bias_red.reshape(OB, P).T,
            np.zeros((P, 1)),
        ],
        axis=1,
    ).astype(np.float32)
    return w, sadd


def _pretranspose(x_shard, sgroups=SGROUPS, cfg=None):
    cfg = dict(CONFIG if cfg is None else cfg)
    mm_np = _np_dt(cfg["mm_dt"])
    x5 = x_shard.astype(mm_np).reshape(sgroups, SGW, IN_DIM)
    return np.ascontiguousarray(x5.transpose(0, 2, 1))


def kernel(x, theta, phi, **run_kwargs):
    from concourse import bass_utils

    nc = _get_nc()
    w, sadd = _host_params(theta, phi)

    in_maps = [
        {
            "xt": _pretranspose(x[c * BS : (c + 1) * BS]),
            "w": w,
            "sadd": sadd,
        }
        for c in range(N_CORES)
    ]
    res = None
    for attempt in range(3):
        try:
            res = bass_utils.run_bass_kernel_spmd(
                nc, in_maps, core_ids=list(range(N_CORES)), **run_kwargs
            )
            break
        except Exception:
            # transient NRT_EXEC_UNIT_UNRECOVERABLE wedges have been observed
            # when a prior process was still tearing down; back off and retry
            if attempt == 2:
                raise
            import time

            time.sleep(15)
    # yt[ob, gs, p, s*GW+b] -> y[(gs*SG+s)*GW+b, ob*P+p]
    y = np.concatenate(
        [
            res.results[c]["yt"].transpose(1, 3, 0, 2).reshape(BS, OUT_DIM)
            for c in range(N_CORES)
        ],
        axis=0,
    ).astype(np.float32)
    if run_kwargs:
        _cache["last_results"] = res
    return y
